# revision 1
# baseline (speedup 1.0000x reference)
"""Trainium2 Bass kernel for nn_CrossLITFusion (sparse window attention fusion).

Self-contained: hardcodes all shapes. Shards the query-pixel axis across the
8 NeuronCores (16 query rows / 8 feature rows + 3-row halo per core).
"""
import sys

sys.path.insert(0, "/opt/trn_rl_repo")

from contextlib import ExitStack

import ml_dtypes
import numpy as np

import concourse.bass as bass
import concourse.bacc as bacc
import concourse.mybir as mybir
import concourse.tile as tile

BF = ml_dtypes.bfloat16
dt = mybir.dt
AF = mybir.ActivationFunctionType
ALU = mybir.AluOpType

# Problem constants
DIM = 64
HEAD = 2
DH = 32
R = 3
HF, WF = 64, 64
HQ, WQ = 128, 128
HID = 256
N_CORES = 8

# Per-core geometry
QROWS = 16             # query rows per core
FROWS = 8              # feature rows owned
PROWS = FROWS + 2 * R  # 14 padded feature rows (halo)
PW = WF + 2 * R        # 70 padded feature cols
DTR = QROWS + 2        # 18 depth input rows
DTW = WQ + 2           # 130 padded depth cols
FTR = PROWS + 2        # 16 feat input rows
FTW = WF + 2           # 66 padded feat cols
NBLK = WF // 2         # 32 col-blocks (2 feat cols each)
BKP = 8 * PROWS        # 112 kpix per block window
NSB = 8                # superblocks (4 blocks each)
QCORE = QROWS * WQ     # 2048 queries per core

_CACHE = {}

# attention pipeline orders queries block-major: qa = 64*b + 4*qy + qxl
# (b = feat col-block = qx//4). _QPERM[qa] = row-major index 128*qy + 4*b + qxl.
_QA = np.arange(QCORE)
_QPERM = 128 * ((_QA % 64) // 4) + 4 * (_QA // 64) + (_QA % 4)


def _build_program(dbg=False):
    nc = bacc.Bacc("TRN2", target_bir_lowering=False, debug=False)

    # ---- DRAM I/O ----
    depth_t = nc.dram_tensor("depth_t", [64, DTR, WQ], dt.bfloat16, kind="ExternalInput").ap()
    feat_t = nc.dram_tensor("feat_t", [64, FTR, WF], dt.bfloat16, kind="ExternalInput").ap()
    depth_res = nc.dram_tensor("depth_res", [QCORE, 64], dt.float32, kind="ExternalInput").ap()
    rowmask = nc.dram_tensor("rowmask", [64, PROWS, WF], dt.bfloat16, kind="ExternalInput").ap()
    wq_pair = nc.dram_tensor("wq_pair", [128, 3, 64], dt.bfloat16, kind="ExternalInput").ap()
    wq_r2 = nc.dram_tensor("wq_r2", [64, 3, 64], dt.bfloat16, kind="ExternalInput").ap()
    wkv_pair = nc.dram_tensor("wkv_pair", [128, 3, 128], dt.bfloat16, kind="ExternalInput").ap()
    wkv_r2 = nc.dram_tensor("wkv_r2", [64, 3, 128], dt.bfloat16, kind="ExternalInput").ap()
    bq_s = nc.dram_tensor("bq_s", [64, 1], dt.float32, kind="ExternalInput").ap()
    bk_in = nc.dram_tensor("bk_in", [64, 1], dt.float32, kind="ExternalInput").ap()
    bv_in = nc.dram_tensor("bv_in", [64, 1], dt.float32, kind="ExternalInput").ap()
    ident_in = nc.dram_tensor("ident_in", [64, 64], dt.bfloat16, kind="ExternalInput").ap()
    expt4 = nc.dram_tensor("expt4", [BKP, 512], dt.bfloat16, kind="ExternalInput").ap()
    w1aug = nc.dram_tensor("w1aug", [97, HID], dt.bfloat16, kind="ExternalInput").ap()
    w2t = nc.dram_tensor("w2t", [128, 2, 64], dt.bfloat16, kind="ExternalInput").ap()
    out_d = nc.dram_tensor("out", [QCORE, 64], dt.float32, kind="ExternalOutput").ap()
    if dbg:
        dbg_qmap = nc.dram_tensor("dbg_qmap", [64, QROWS, DTW], dt.float32, kind="ExternalOutput").ap()
        dbg_kv = nc.dram_tensor("dbg_kv", [128, PW, PROWS], dt.float32, kind="ExternalOutput").ap()
        dbg_pw = nc.dram_tensor("dbg_pw", [BKP, NSB, 512], dt.float32, kind="ExternalOutput").ap()
        dbg_attn = nc.dram_tensor("dbg_attn", [97, QCORE], dt.float32, kind="ExternalOutput").ap()
        dbg_sraw = nc.dram_tensor("dbg_sraw", [33, QCORE], dt.float32, kind="ExternalOutput").ap()
        dbg_ht = nc.dram_tensor("dbg_ht", [128, 16, HID], dt.float32, kind="ExternalOutput").ap()

    with tile.TileContext(nc) as tc, ExitStack() as ctx:
        const = ctx.enter_context(tc.tile_pool(name="const", bufs=1))
        maps = ctx.enter_context(tc.tile_pool(name="maps", bufs=1))

        # ---- load constants ----
        wqp = const.tile([128, 3, 64], dt.bfloat16)
        nc.sync.dma_start(wqp[:], wq_pair)
        wq2t = const.tile([128, 3, 64], dt.bfloat16)
        nc.sync.dma_start(wq2t[64:128, :, :], wq_r2)
        wkvp = const.tile([128, 3, 128], dt.bfloat16)
        nc.sync.dma_start(wkvp[:], wkv_pair)
        wkv2t = const.tile([128, 3, 128], dt.bfloat16)
        nc.sync.dma_start(wkv2t[64:128, :, :], wkv_r2)
        bq = const.tile([64, 1], dt.float32)
        nc.sync.dma_start(bq[:], bq_s)
        bkv = const.tile([128, 1], dt.float32)
        nc.sync.dma_start(bkv[0:64, :], bk_in)
        nc.sync.dma_start(bkv[64:128, :], bv_in)
        identt = const.tile([128, 64], dt.bfloat16)
        nc.sync.dma_start(identt[64:128, :], ident_in)
        expt = const.tile([BKP, 512], dt.bfloat16)
        nc.sync.dma_start(expt[:], expt4)
        w1a = const.tile([97, HID], dt.bfloat16)
        nc.sync.dma_start(w1a[:], w1aug)
        w2 = const.tile([128, 2, 64], dt.bfloat16)
        nc.sync.dma_start(w2[:], w2t)
        mask = const.tile([128, PROWS, WF], dt.bfloat16)
        nc.sync.dma_start(mask[0:64, :, :], rowmask)
        nc.sync.dma_start(mask[64:128, :, :], rowmask)
        dres = const.tile([128, 16, 64], dt.float32)
        nc.sync.dma_start(dres[:], depth_res.rearrange("(i p) c -> p i c", p=128))

        # ---- build doubled conv input tiles ----
        d2 = maps.tile([128, DTR, DTW], dt.bfloat16)
        nc.gpsimd.memset(d2[:], 0.0)
        nc.sync.dma_start(d2[0:64, :, 1 : 1 + WQ], depth_t)
        d2f = d2[:].rearrange("c a b -> c (a b)")
        nc.gpsimd.dma_start(d2f[64:128, 0 : DTR * DTW - DTW], d2f[0:64, DTW : DTR * DTW])

        f2 = maps.tile([128, FTR, FTW], dt.bfloat16)
        nc.gpsimd.memset(f2[:], 0.0)
        nc.sync.dma_start(f2[0:64, :, 1 : 1 + WF], feat_t)
        f2f = f2[:].rearrange("c a b -> c (a b)")
        nc.gpsimd.dma_start(f2f[64:128, 0 : FTR * FTW - FTW], f2f[0:64, FTW : FTR * FTW])

        # ---- conv_q -> Qmap (64, 16, 130) bf16, scaled by 1/sqrt(dh) ----
        qmap = maps.tile([64, QROWS, DTW], dt.bfloat16)
        nc.gpsimd.memset(qmap[:], 0.0)
        qmapf = qmap[:].rearrange("c a b -> c (a b)")
        chunks = [(1, 416), (417, 416), (833, 416), (1249, 416), (1665, 414)]
        with tc.tile_pool(name="cq_ps", bufs=2, space="PSUM") as cq_ps:
            for (o0, n) in chunks:
                acc = cq_ps.tile([64, 416], dt.float32, tag="cq")
                for kx in range(3):
                    nc.tensor.matmul(acc[:, 0:n], wqp[:, kx, :],
                                     d2f[:, o0 + kx - 1 : o0 + kx - 1 + n],
                                     start=(kx == 0), stop=False)
                for kx in range(3):
                    nc.tensor.matmul(acc[:, 0:n], wq2t[64:128, kx, :],
                                     d2f[64:128, o0 + DTW + kx - 1 : o0 + DTW + kx - 1 + n],
                                     start=False, stop=(kx == 2))
                nc.scalar.activation(qmapf[:, o0 : o0 + n], acc[:, 0:n], AF.Identity,
                                     bias=bq[:], scale=float(1.0 / np.sqrt(DH)))

        # ---- Q2: block-major zero-interleaved Q (64, NBLK, 128) ----
        # per block b: cols 0-63 = head0 q (qy*4+qxl), cols 64-127 = head1 q
        q2 = maps.tile([64, NBLK, 128], dt.bfloat16)
        nc.gpsimd.memset(q2[:], 0.0)
        for y in range(QROWS):
            qsrc = qmap[:, y, 1 : 1 + WQ].rearrange("c (b q) -> c b q", b=NBLK)
            nc.gpsimd.dma_start(q2[0:32, :, 4 * y : 4 * y + 4], qsrc[0:32])
            nc.gpsimd.dma_start(q2[32:64, :, 64 + 4 * y : 64 + 4 * y + 4], qsrc[32:64])

        # ---- conv_k + conv_v fused -> KV x-major (128, 70, 14): K rows 0-63, V 64-127
        # kv[c, x, y]: kpix index = x*14 + y (column-major over the padded map)
        kv = maps.tile([128, PW, PROWS], dt.bfloat16)
        nc.gpsimd.memset(kv[:], 0.0)
        with tc.tile_pool(name="kv_ps", bufs=2, space="PSUM") as kv_ps:
            for half in range(2):
                py0 = half * 7
                o0 = py0 * FTW + 1
                acc = kv_ps.tile([128, 7, FTW], dt.float32, tag="kv")
                accf = acc[:].rearrange("c y x -> c (y x)")
                for kx in range(3):
                    nc.tensor.matmul(accf[:, 1:461], wkvp[:, kx, :],
                                     f2f[:, o0 + kx - 1 : o0 + kx - 1 + 460],
                                     start=(kx == 0), stop=False)
                for kx in range(3):
                    nc.tensor.matmul(accf[:, 1:461], wkv2t[64:128, kx, :],
                                     f2f[64:128, o0 + FTW + kx - 1 : o0 + FTW + kx - 1 + 460],
                                     start=False, stop=(kx == 2))
                # (conv + bias) * row-validity mask, downcast + scatter to x-major
                accv = acc[:]
                kvdst = kv[:, R : R + WF, py0 : py0 + 7].rearrange("c x y -> c y x")
                nc.vector.scalar_tensor_tensor(
                    kvdst[0:64], accv[0:64, 0:7, 1 : 1 + WF], bkv[0:64, :],
                    mask[0:64, py0 : py0 + 7, :], op0=ALU.add, op1=ALU.mult)
                nc.vector.scalar_tensor_tensor(
                    kvdst[64:128], accv[64:128, 0:7, 1 : 1 + WF], bkv[64:128, :],
                    mask[64:128, py0 : py0 + 7, :], op0=ALU.add, op1=ALU.mult)

        # ---- V_T tiles (112, 97) per block: cols [Vh0 |1| 0*31 | Vh1 |1] ----
        vt_all = maps.tile([BKP, NBLK * 97], dt.bfloat16)
        nc.gpsimd.memset(vt_all[:], 0.0)
        vt_v = vt_all[:].rearrange("p (b m) -> p b m", m=97)
        nc.gpsimd.memset(vt_v[:, :, 32:33], 1.0)
        nc.gpsimd.memset(vt_v[:, :, 96:97], 1.0)
        with tc.tile_pool(name="vt_ps", bufs=3, space="PSUM") as vt_ps:
            for b in range(NBLK):
                vin = kv[64:128, 2 * b : 2 * b + 8, :]
                tp = vt_ps.tile([BKP, 64], dt.bfloat16, tag="vt")
                nc.tensor.transpose(tp[:], vin, identt[64:128, :])
                nc.vector.tensor_copy(vt_all[:, 97 * b : 97 * b + 32], tp[:, 0:32])
                nc.vector.tensor_copy(vt_all[:, 97 * b + 64 : 97 * b + 96], tp[:, 32:64])

        # ---- attention ----
        attn = maps.tile([97, QCORE], dt.bfloat16)
        nc.gpsimd.memset(attn[:], 0.0)
        nc.gpsimd.memset(attn[32:33, :], 1.0)
        s_raw = maps.tile([33, QCORE], dt.float32)
        s_t = maps.tile([128, 2, 2 * NSB], dt.float32)
        s_tr = maps.tile([128, 2, 2 * NSB], dt.float32)
        s_r = maps.tile([2, QCORE], dt.float32)
        bcast = maps.tile([96, QCORE], dt.float32)

        with tc.tile_pool(name="qk_ps", bufs=2, space="PSUM") as qk_ps, \
             tc.tile_pool(name="av_ps", bufs=3, space="PSUM") as av_ps, \
             tc.tile_pool(name="p_pool", bufs=2) as p_pool:
            for sb in range(NSB):
                qkp = qk_ps.tile([BKP, 512], dt.float32, tag="qk")
                for j in range(4):
                    b = 4 * sb + j
                    klhs = kv[0:64, 2 * b : 2 * b + 8, :]
                    qrhs = q2[:, b, :]
                    nc.tensor.matmul(qkp[:, 128 * j : 128 * j + 128], klhs, qrhs,
                                     start=True, stop=True)
                pex = p_pool.tile([BKP, 512], dt.bfloat16, tag="pex")
                nc.scalar.activation(pex[:], qkp[:], AF.Exp)
                pw = p_pool.tile([BKP, 512], dt.bfloat16, tag="pw")
                nc.gpsimd.tensor_mul(pw[:], pex[:], expt[:])
                if dbg:
                    dpw = p_pool.tile([BKP, 512], dt.float32, tag="dpw")
                    nc.vector.tensor_copy(dpw[:], pw[:])
                    nc.sync.dma_start(dbg_pw[:, sb, :], dpw[:])
                avp = av_ps.tile([97, 4, 128], dt.float32, tag="av")
                for j in range(4):
                    b = 4 * sb + j
                    nc.tensor.matmul(avp[:, j, :], vt_all[:, 97 * b : 97 * b + 97],
                                     pw[:, 128 * j : 128 * j + 128], start=True, stop=True)
                # softmax denominators -> reciprocal -> broadcast along partitions
                sr0 = s_raw[0:1, 256 * sb : 256 * sb + 256].rearrange("p (j c) -> p j c", j=4)
                sr1 = s_raw[32:33, 256 * sb : 256 * sb + 256].rearrange("p (j c) -> p j c", j=4)
                nc.vector.tensor_copy(sr0, avp[32:33, :, 0:64])
                nc.vector.tensor_copy(sr1, avp[96:97, :, 64:128])
                nc.sync.dma_start(s_t[:, 0, 2 * sb : 2 * sb + 2],
                                  s_raw[0:1, 256 * sb : 256 * sb + 256])
                nc.sync.dma_start(s_t[:, 1, 2 * sb : 2 * sb + 2],
                                  s_raw[32:33, 256 * sb : 256 * sb + 256])
                nc.vector.reciprocal(s_tr[:, :, 2 * sb : 2 * sb + 2],
                                     s_t[:, :, 2 * sb : 2 * sb + 2])
                nc.sync.dma_start(s_r[0:1, 256 * sb : 256 * sb + 256],
                                  s_tr[:, 0, 2 * sb : 2 * sb + 2])
                nc.sync.dma_start(s_r[1:2, 256 * sb : 256 * sb + 256],
                                  s_tr[:, 1, 2 * sb : 2 * sb + 2])
                for h in range(2):
                    src = s_r[h : h + 1, 256 * sb : 256 * sb + 256]
                    rb = bass.AP(src.tensor, src.offset,
                                 [list(src.ap[0]), [0, 32], [1, 256]])
                    nc.sync.dma_start(bcast[64 * h : 64 * h + 32,
                                            256 * sb : 256 * sb + 256], rb)
                # normalize -> attn tile (bf16)
                a0 = attn[0:32, 256 * sb : 256 * sb + 256].rearrange("p (j c) -> p j c", j=4)
                a1 = attn[64:96, 256 * sb : 256 * sb + 256].rearrange("p (j c) -> p j c", j=4)
                b0 = bcast[0:32, 256 * sb : 256 * sb + 256].rearrange("p (j c) -> p j c", j=4)
                b1 = bcast[64:96, 256 * sb : 256 * sb + 256].rearrange("p (j c) -> p j c", j=4)
                nc.vector.tensor_mul(a0, avp[0:32, :, 0:64], b0)
                nc.vector.tensor_mul(a1, avp[64:96, :, 64:128], b1)

        if dbg:
            dqm = maps.tile([64, QROWS, DTW], dt.float32)
            nc.vector.tensor_copy(dqm[:], qmap[:])
            nc.sync.dma_start(dbg_qmap, dqm[:])
            dkv = maps.tile([128, PW, PROWS], dt.float32)
            nc.vector.tensor_copy(dkv[:], kv[:])
            nc.sync.dma_start(dbg_kv, dkv[:])
            datn = maps.tile([97, QCORE], dt.float32)
            nc.vector.tensor_copy(datn[:], attn[:])
            nc.sync.dma_start(dbg_attn, datn[:])
            nc.sync.dma_start(dbg_sraw[0:1, :], s_raw[0:1, :])
            nc.sync.dma_start(dbg_sraw[32:33, :], s_raw[32:33, :])

        # ---- MLP1 (pixel-major) + gelu -> h_T, DMA-transpose -> h_cm ----
        h_t = maps.tile([128, 16, HID], dt.bfloat16)
        h_cm = maps.tile([128, 2, QCORE], dt.bfloat16)
        with tc.tile_pool(name="m1_ps", bufs=2, space="PSUM") as m1_ps:
            for i in range(16):
                acc = m1_ps.tile([128, HID], dt.float32, tag="m1")
                nc.tensor.matmul(acc[:], attn[0:97, 128 * i : 128 * i + 128], w1a[:],
                                 start=True, stop=True)
                nc.scalar.activation(h_t[:, i, :], acc[:], AF.Gelu)
                if dbg:
                    dht = maps.tile([128, HID], dt.float32, tag="dht")
                    nc.vector.tensor_copy(dht[:], h_t[:, i, :])
                    nc.sync.dma_start(dbg_ht[:, i, :], dht[:])
                for g in range(2):
                    nc.sync.dma_start(h_cm[:, g, 128 * i : 128 * i + 128],
                                      h_t[:, i, 128 * g : 128 * g + 128], transpose=True)

        # ---- MLP2 + residual ----
        out_r = out_d.rearrange("(i p) c -> p i c", p=128)
        with tc.tile_pool(name="m2_ps", bufs=2, space="PSUM") as m2_ps, \
             tc.tile_pool(name="o_pool", bufs=2) as o_pool:
            for i in range(16):
                acc = m2_ps.tile([128, 64], dt.float32, tag="m2")
                nc.tensor.matmul(acc[:], h_cm[:, 0, 128 * i : 128 * i + 128], w2[:, 0, :],
                                 start=True, stop=False)
                nc.tensor.matmul(acc[:], h_cm[:, 1, 128 * i : 128 * i + 128], w2[:, 1, :],
                                 start=False, stop=True)
                ot = o_pool.tile([128, 64], dt.float32, tag="ot")
                nc.vector.tensor_add(ot[:], acc[:], dres[:, i, :])
                nc.sync.dma_start(out_r[:, i, :], ot[:])

    nc.compile()
    return nc


def _host_prep(depth, x, cell, conv_q_w, conv_q_b, conv_k_w, conv_k_b,
               conv_v_w, conv_v_b, cpb_w1, cpb_b1, cpb_w2,
               mlp_w1, mlp_b1, mlp_w2, mlp_b2):
    """Build the 8 per-core input maps."""
    f32 = np.float32
    depth = np.asarray(depth, f32)
    x = np.asarray(x, f32)
    cell = np.asarray(cell, f32)

    depth_T = np.ascontiguousarray(depth[0].T).reshape(64, HQ, WQ)
    feat_T = np.ascontiguousarray(x[0].T).reshape(64, HF, WF)

    wq = np.asarray(conv_q_w, f32)
    wk = np.asarray(conv_k_w, f32)
    wv = np.asarray(conv_v_w, f32)
    wq_pair = np.stack([np.concatenate([wq[:, :, 0, kx].T, wq[:, :, 1, kx].T], 0)
                        for kx in range(3)], axis=1).astype(BF)
    wq_r2 = np.stack([wq[:, :, 2, kx].T for kx in range(3)], axis=1).astype(BF)

    def kv_lhs(ky, kx):
        return np.concatenate([wk[:, :, ky, kx].T, wv[:, :, ky, kx].T], 1)

    wkv_pair = np.stack([np.concatenate([kv_lhs(0, kx), kv_lhs(1, kx)], 0)
                         for kx in range(3)], axis=1).astype(BF)
    wkv_r2 = np.stack([kv_lhs(2, kx) for kx in range(3)], axis=1).astype(BF)

    bq_np = (np.asarray(conv_q_b, f32) / np.sqrt(f32(DH))).reshape(64, 1).astype(f32)
    bk = np.asarray(conv_k_b, f32).reshape(64, 1).astype(f32)
    bv = np.asarray(conv_v_b, f32).reshape(64, 1).astype(f32)
    ident = np.eye(64, dtype=f32).astype(BF)

    # position-bias table -> multiplicative exp table with window mask
    w1 = np.asarray(cpb_w1, f32)
    b1 = np.asarray(cpb_b1, f32)
    w2c = np.asarray(cpb_w2, f32)
    dy = (np.linspace(-R, R, 2 * R + 1).astype(f32)) * f32(2.0 / HF)
    delta = np.stack(np.meshgrid(dy, dy, indexing="ij"), -1).reshape(-1, 2)
    pb = np.zeros((HEAD, 2, 2, 7, 7), f32)
    for iy in range(2):
        for jx in range(2):
            sy = f32(-1.0) + (2 * iy + 1) / f32(HQ)
            sx = f32(-1.0) + (2 * jx + 1) / f32(WQ)
            base = np.array([f32(-1.0) + 1.0 / f32(HF), f32(-1.0) + 1.0 / f32(WF)], f32)
            ck = base[None, :] + delta
            rel = (np.array([sy, sx], f32)[None, :] - ck) * np.array([HQ, WQ], f32)
            h = np.maximum(rel @ w1.T + b1, 0.0)
            p = h @ w2c.T
            pb[:, iy, jx] = p.T.reshape(HEAD, 7, 7)

    expt = np.zeros((BKP, 128), f32)
    for kp in range(BKP):
        xk, py = kp // PROWS, kp % PROWS
        for n in range(128):
            h, r = n // 64, n % 64
            qy, qx = r // 4, r % 4
            dyy = py - (qy // 2 + R)
            dxx = xk - (qx // 2 + R)
            if abs(dyy) <= R and abs(dxx) <= R:
                expt[kp, n] = np.exp(pb[h, qy % 2, qx % 2, dyy + R, dxx + R])
    expt4 = np.tile(expt, (1, 4)).astype(BF)

    m1w = np.asarray(mlp_w1, f32)
    m1b = np.asarray(mlp_b1, f32)
    m2w = np.asarray(mlp_w2, f32)
    m2b = np.asarray(mlp_b2, f32)
    rel_cell = cell[0] * np.array([HF, WF], f32)
    b1pp = m1b + m1w[:, 64:66] @ rel_cell
    w1aug = np.zeros((97, HID), f32)
    w1aug[0:32] = m1w[:, 0:32].T
    w1aug[32] = b1pp
    w1aug[64:96] = m1w[:, 32:64].T
    w1aug = w1aug.astype(BF)
    w2t = np.stack([m2w[:, 0:128].T, m2w[:, 128:256].T], axis=1).astype(BF)

    in_maps = []
    shared = dict(wq_pair=wq_pair, wq_r2=wq_r2, wkv_pair=wkv_pair, wkv_r2=wkv_r2,
                  bq_s=bq_np, bk_in=bk, bv_in=bv, ident_in=ident, expt4=expt4,
                  w1aug=w1aug, w2t=w2t)
    for t in range(N_CORES):
        dslab = np.zeros((64, DTR, WQ), f32)
        g0 = QROWS * t - 1
        lo, hi = max(0, g0), min(HQ, g0 + DTR)
        dslab[:, lo - g0 : hi - g0, :] = depth_T[:, lo:hi, :]
        fslab = np.zeros((64, FTR, WF), f32)
        f0 = FROWS * t - R - 1
        flo, fhi = max(0, f0), min(HF, f0 + FTR)
        fslab[:, flo - f0 : fhi - f0, :] = feat_T[:, flo:fhi, :]
        mrow = np.zeros((PROWS,), f32)
        for py in range(PROWS):
            g = FROWS * t - R + py
            if 0 <= g < HF:
                mrow[py] = 1.0
        rowmask_np = np.broadcast_to(mrow[None, :, None], (64, PROWS, WF)).astype(BF)
        dres_rows = depth[0, QCORE * t : QCORE * (t + 1), :] + m2b[None, :]
        dres_np = dres_rows[_QPERM].astype(f32)
        m = dict(shared)
        m.update(depth_t=dslab.astype(BF), feat_t=fslab.astype(BF),
                 depth_res=np.ascontiguousarray(dres_np),
                 rowmask=np.ascontiguousarray(rowmask_np))
        in_maps.append(m)
    return in_maps


LAST_RESULT = None


def _prep_inputs(inputs):
    keys = dict(inputs)
    keys.pop("shape_y", None)
    keys.pop("shape_x", None)
    return _host_prep(**keys)


def kernel(**inputs):
    global LAST_RESULT
    from concourse.bass_utils import run_bass_kernel_spmd

    if "nc" not in _CACHE:
        _CACHE["nc"] = _build_program()
    nc = _CACHE["nc"]
    in_maps = _prep_inputs(inputs)
    res = run_bass_kernel_spmd(nc, in_maps, core_ids=list(range(N_CORES)))
    LAST_RESULT = res
    parts = []
    for t in range(N_CORES):
        oc = res.results[t]["out"]
        orow = np.empty_like(oc)
        orow[_QPERM] = oc
        parts.append(orow)
    out = np.concatenate(parts, 0)
    return out[None].astype(np.float32)


def _patch_sim_gelu():
    import math
    import concourse.bass_interp as bi
    import concourse.mybir as mb
    if getattr(bi.InstructionExecutor, "_gelu_patched", False):
        return
    orig = bi.InstructionExecutor.visit_InstActivation
    from concourse.bass_interp import Direction

    erf = np.vectorize(math.erf)

    def patched(self, instruction, *, reg_snapshot=None):
        if getattr(instruction, "func", None) == mb.ActivationFunctionType.Gelu:
            instruction.func = mb.ActivationFunctionType.Identity
            try:
                r = orig(self, instruction, reg_snapshot=reg_snapshot)
            finally:
                instruction.func = mb.ActivationFunctionType.Gelu
            ov = self.view_ap(instruction.outs[0], Direction.WRITE, instruction,
                              reg_snapshot=reg_snapshot)
            x = np.asarray(ov[:], dtype=np.float64)
            ov[:] = (0.5 * x * (1.0 + erf(x / np.sqrt(2.0)))).astype(np.float32)
            return r
        return orig(self, instruction, reg_snapshot=reg_snapshot)

    bi.InstructionExecutor.visit_InstActivation = patched
    bi.InstructionExecutor._gelu_patched = True


def simulate_core(core=0, inputs=None, dbg=False):
    """CoreSim single-core check helper (dev only)."""
    from concourse.bass_interp import CoreSim

    _patch_sim_gelu()

    key = f"nc_dbg{dbg}"
    if key not in _CACHE:
        _CACHE[key] = _build_program(dbg=dbg)
    nc = _CACHE[key]
    in_maps = _prep_inputs(inputs)
    sim = CoreSim(nc, trace=False)
    for k, v in in_maps[core].items():
        sim.tensor(k)[:] = v
    sim.simulate(check_with_hw=False)
    if dbg:
        names = ["out", "dbg_qmap", "dbg_kv", "dbg_pw", "dbg_attn", "dbg_sraw", "dbg_ht"]
        return {n: np.array(sim.tensor(n)) for n in names}
    return np.array(sim.tensor("out"))



# revision 3
# speedup vs baseline: 2.0017x; 2.0017x over previous
"""Trainium2 Bass kernel for nn_CrossLITFusion (sparse window attention fusion).

Self-contained: hardcodes all shapes. Shards the query-pixel axis across the
8 NeuronCores (16 query rows / 8 feature rows + 3-row halo per core).

v2: host-side layout prep (block-major depth, x-major feat), transposed MLP1
(no DMA transposes), in-matmul softmax denominators, batched DMAs.
"""
import sys

sys.path.insert(0, "/opt/trn_rl_repo")

from contextlib import ExitStack

import ml_dtypes
import numpy as np

import concourse.bass as bass
import concourse.bacc as bacc
import concourse.mybir as mybir
import concourse.tile as tile

BF = ml_dtypes.bfloat16
dt = mybir.dt
AF = mybir.ActivationFunctionType
ALU = mybir.AluOpType

# Problem constants
DIM = 64
HEAD = 2
DH = 32
R = 3
HF, WF = 64, 64
HQ, WQ = 128, 128
HID = 256
N_CORES = 8

QROWS = 16             # query rows per core
FROWS = 8              # feature rows owned
PROWS = FROWS + 2 * R  # 14 padded feature rows (halo)
PW = WF + 2 * R        # 70 padded feature cols
NBLK = WF // 2         # 32 col-blocks (4 query cols each)
BKP = 8 * PROWS        # 112 kpix per block window
NSB = 8                # superblocks (4 blocks each)
QCORE = QROWS * WQ     # 2048 queries per core

# blob16 column offsets
C_WQP = 0       # [128, 3, 64]
C_WQ2 = 192     # [64:128, 3, 64]
C_WKVP = 384    # [128, 3, 128]
C_WKV2 = 768    # [64:128, 3, 128]
C_ID = 1152     # [64:128, 64]
C_W1 = 1216     # [0:65, 256]
C_W2 = 1472     # [128, 2, 64]
C_MSK = 1600    # [128, 32*14]
C_EXP = 2048    # [0:112, 512]
NB16 = 2560

_CACHE = {}

# attention pipeline orders queries block-major: qa = 64*b + 4*qy + qxl
# (b = feat col-block = qx//4). _QPERM[qa] = row-major index 128*qy + 4*b + qxl.
_QA = np.arange(QCORE)
_QPERM = 128 * ((_QA % 64) // 4) + 4 * (_QA // 64) + (_QA % 4)


def _build_program():
    nc = bacc.Bacc("TRN2", target_bir_lowering=False, debug=False)

    # ---- DRAM I/O ----
    blob16_d = nc.dram_tensor("blob16", [128, NB16], dt.bfloat16, kind="ExternalInput").ap()
    blob32_d = nc.dram_tensor("blob32", [128, 2], dt.float32, kind="ExternalInput").ap()
    d2_d = nc.dram_tensor("d2blk", [128, NBLK, 17, 6], dt.bfloat16, kind="ExternalInput").ap()
    f2_d = nc.dram_tensor("f2d", [128, PW, 16], dt.bfloat16, kind="ExternalInput").ap()
    dres_d = nc.dram_tensor("dres3", [128, 16, 64], dt.float32, kind="ExternalInput").ap()
    out_d = nc.dram_tensor("out", [128, 16, 64], dt.float32, kind="ExternalOutput").ap()

    with tile.TileContext(nc) as tc, ExitStack() as ctx:
        const = ctx.enter_context(tc.tile_pool(name="const", bufs=1))
        maps = ctx.enter_context(tc.tile_pool(name="maps", bufs=1))

        # ---- SBUF tiles ----
        cb16 = const.tile([128, NB16], dt.bfloat16)
        cb32 = const.tile([128, 2], dt.float32)
        d2 = maps.tile([128, NBLK, 17, 6], dt.bfloat16)
        f2 = maps.tile([128, PW, 16], dt.bfloat16)
        dresT = maps.tile([128, 16, 64], dt.float32)
        qmap = maps.tile([64, NBLK, 64], dt.bfloat16)
        q2 = maps.tile([64, NBLK, 128], dt.bfloat16)
        kv = maps.tile([128, PW, PROWS], dt.bfloat16)
        vt = maps.tile([BKP, NBLK, 96], dt.bfloat16)
        attn = maps.tile([65, QCORE], dt.bfloat16)
        hg = maps.tile([128, 2, QCORE], dt.bfloat16)
        ot = maps.tile([128, 16, 64], dt.float32)

        # ---- input DMAs: sync queue ----
        nc.sync.dma_start(cb16[:], blob16_d)
        nc.sync.dma_start(cb32[:], blob32_d)
        nc.sync.dma_start(d2[:], d2_d)
        nc.sync.dma_start(dresT[:], dres_d)
        # scalar queue
        nc.scalar.dma_start(f2[:], f2_d)

        # ---- const views ----
        wqp = cb16[:, C_WQP : C_WQP + 192].rearrange("c (k m) -> c k m", k=3)
        wq2 = cb16[64:128, C_WQ2 : C_WQ2 + 192].rearrange("c (k m) -> c k m", k=3)
        wkvp = cb16[:, C_WKVP : C_WKVP + 384].rearrange("c (k m) -> c k m", k=3)
        wkv2 = cb16[64:128, C_WKV2 : C_WKV2 + 384].rearrange("c (k m) -> c k m", k=3)
        ident = cb16[64:128, C_ID : C_ID + 64]
        w1a = cb16[0:65, C_W1 : C_W1 + 256]
        w2 = cb16[:, C_W2 : C_W2 + 128].rearrange("c (g m) -> c g m", g=2)
        msk = cb16[:, C_MSK : C_MSK + 448].rearrange("c (x y) -> c x y", x=32)
        expt = cb16[0:112, C_EXP : C_EXP + 512]
        bq = cb32[0:64, 0:1]
        bkv = cb32[:, 1:2]

        # ---- memsets (off critical path) ----
        nc.vector.memset(q2[0:32, :, 64:128], 0.0)
        nc.gpsimd.memset(q2[32:64, :, 0:64], 0.0)
        nc.vector.memset(kv[:, 0:R, :], 0.0)
        nc.vector.memset(kv[:, R + WF : PW, :], 0.0)
        nc.vector.memset(vt[:, :, 64:96], 1.0)
        nc.vector.memset(attn[64:65, :], 1.0)

        # ---- conv_k + conv_v fused -> KV x-major (128, 70, 14) ----
        # kv[c, x, y]: K channels rows 0-63, V rows 64-127
        with tc.tile_pool(name="kv_ps", bufs=2, space="PSUM") as kv_ps:
            for xc in range(2):
                x0 = R + 32 * xc
                acc = kv_ps.tile([128, 32, PROWS], dt.float32, tag="kv")
                for ky in range(3):
                    nc.tensor.matmul(acc[:], wkvp[:, ky, :],
                                     f2[:, x0 - 1 : x0 + 31, ky : ky + PROWS],
                                     start=(ky == 0), stop=False)
                for ky in range(3):
                    nc.tensor.matmul(acc[:], wkv2[:, ky, :],
                                     f2[64:128, x0 : x0 + 32, ky : ky + PROWS],
                                     start=False, stop=(ky == 2))
                # (conv + bias) * row-validity mask
                nc.vector.scalar_tensor_tensor(
                    kv[0:64, x0 : x0 + 32, :], acc[0:64], bkv[0:64],
                    msk[0:64], op0=ALU.add, op1=ALU.mult)
                nc.vector.scalar_tensor_tensor(
                    kv[64:128, x0 : x0 + 32, :], acc[64:128], bkv[64:128],
                    msk[64:128], op0=ALU.add, op1=ALU.mult)

        # ---- V_T tiles (112, 96) per block: [Vh0(32) | Vh1(32) | ones(32)] ----
        with tc.tile_pool(name="vt_ps", bufs=3, space="PSUM") as vt_ps:
            for b in range(NBLK):
                tp = vt_ps.tile([BKP, 64], dt.bfloat16, tag="vt")
                nc.tensor.transpose(tp[:], kv[64:128, 2 * b : 2 * b + 8, :], ident)
                nc.scalar.activation(vt[:, b, 0:64], tp[:], AF.Identity)

        # ---- main pipeline: conv_q chunks + attention + MLP ----
        qsc = float(1.0 / np.sqrt(DH))
        with tc.tile_pool(name="cq_ps", bufs=2, space="PSUM") as cq_ps, \
             tc.tile_pool(name="sc_ps", bufs=3, space="PSUM") as sc_ps, \
             tc.tile_pool(name="m1_ps", bufs=2, space="PSUM") as m1_ps, \
             tc.tile_pool(name="m2_ps", bufs=1, space="PSUM") as m2_ps, \
             tc.tile_pool(name="p_pool", bufs=2) as p_pool:

            def conv_q_chunk(c):
                b0 = 8 * c
                acc = cq_ps.tile([64, 512], dt.float32, tag="cq")
                accv = acc[:].rearrange("c (b y x) -> c b y x", b=8, y=16)
                for kx in range(3):
                    nc.tensor.matmul(accv, wqp[:, kx, :],
                                     d2[:, b0 : b0 + 8, 0:16, kx : kx + 4],
                                     start=(kx == 0), stop=False)
                for kx in range(3):
                    nc.tensor.matmul(accv, wq2[:, kx, :],
                                     d2[64:128, b0 : b0 + 8, 1:17, kx : kx + 4],
                                     start=False, stop=(kx == 2))
                nc.scalar.activation(qmap[:, b0 : b0 + 8, :], accv, AF.Identity,
                                     bias=bq, scale=qsc)
                nc.sync.dma_start(q2[0:32, b0 : b0 + 8, 0:64],
                                  qmap[0:32, b0 : b0 + 8, :])
                nc.scalar.dma_start(q2[32:64, b0 : b0 + 8, 64:128],
                                    qmap[32:64, b0 : b0 + 8, :])

            def attention_sb(sb):
                qkp = sc_ps.tile([128, 512], dt.float32, tag="sc")
                for j in range(4):
                    b = 4 * sb + j
                    nc.tensor.matmul(qkp[0:112, 128 * j : 128 * j + 128],
                                     kv[0:64, 2 * b : 2 * b + 8, :], q2[:, b, :],
                                     start=True, stop=True)
                pex = p_pool.tile([BKP, 512], dt.bfloat16, tag="pex")
                nc.scalar.activation(pex[:], qkp[0:112, :], AF.Exp)
                pw = p_pool.tile([BKP, 512], dt.bfloat16, tag="pw")
                nc.vector.tensor_mul(pw[:], pex[:], expt)
                avp = sc_ps.tile([128, 4, 128], dt.float32, tag="sc")
                for j in range(4):
                    b = 4 * sb + j
                    nc.tensor.matmul(avp[0:96, j, :], vt[:, b, :],
                                     pw[:, 128 * j : 128 * j + 128],
                                     start=True, stop=True)
                a0 = attn[0:32, 256 * sb : 256 * sb + 256].rearrange(
                    "p (j c) -> p j c", j=4)
                a1 = attn[32:64, 256 * sb : 256 * sb + 256].rearrange(
                    "p (j c) -> p j c", j=4)
                rcp = p_pool.tile([64, 4, 64], dt.float32, tag="rcp")
                nc.vector.reciprocal(rcp[0:32], avp[64:96, :, 0:64])
                nc.vector.reciprocal(rcp[32:64], avp[64:96, :, 64:128])
                nc.vector.tensor_mul(a0, avp[0:32, :, 0:64], rcp[0:32])
                nc.vector.tensor_mul(a1, avp[32:64, :, 64:128], rcp[32:64])

            def mlp_chunk(c):
                for g in range(2):
                    m1 = m1_ps.tile([128, 512], dt.float32, tag="m1")
                    nc.tensor.matmul(m1[:], w1a[:, 128 * g : 128 * g + 128],
                                     attn[:, 512 * c : 512 * c + 512],
                                     start=True, stop=True)
                    nc.scalar.activation(hg[:, g, 512 * c : 512 * c + 512],
                                         m1[:], AF.Gelu)
                for i in range(4 * c, 4 * c + 4):
                    m2 = m2_ps.tile([128, 64], dt.float32, tag="m2")
                    nc.tensor.matmul(m2[:], hg[:, 0, 128 * i : 128 * i + 128],
                                     w2[:, 0, :], start=True, stop=False)
                    nc.tensor.matmul(m2[:], hg[:, 1, 128 * i : 128 * i + 128],
                                     w2[:, 1, :], start=False, stop=True)
                    nc.vector.tensor_add(ot[:, i, :], m2[:], dresT[:, i, :])

            for c in range(4):
                conv_q_chunk(c)
                if c > 0:
                    mlp_chunk(c - 1)
                attention_sb(2 * c)
                attention_sb(2 * c + 1)
            mlp_chunk(3)

        nc.sync.dma_start(out_d, ot[:])

    nc.compile()
    return nc


def _host_prep(depth, x, cell, conv_q_w, conv_q_b, conv_k_w, conv_k_b,
               conv_v_w, conv_v_b, cpb_w1, cpb_b1, cpb_w2,
               mlp_w1, mlp_b1, mlp_w2, mlp_b2):
    """Build the 8 per-core input maps."""
    f32 = np.float32
    depth = np.asarray(depth, f32)
    x = np.asarray(x, f32)
    cell = np.asarray(cell, f32)

    depth_T = np.ascontiguousarray(depth[0].T).reshape(64, HQ, WQ)
    feat_T = np.ascontiguousarray(x[0].T).reshape(64, HF, WF)

    wq = np.asarray(conv_q_w, f32)
    wk = np.asarray(conv_k_w, f32)
    wv = np.asarray(conv_v_w, f32)

    blob16 = np.zeros((128, NB16), f32)
    # wq pair: rows c -> ky=0, rows 64+c -> ky=1, per kx
    for kx in range(3):
        blob16[0:64, C_WQP + 64 * kx : C_WQP + 64 * kx + 64] = wq[:, :, 0, kx].T
        blob16[64:128, C_WQP + 64 * kx : C_WQP + 64 * kx + 64] = wq[:, :, 1, kx].T
        blob16[64:128, C_WQ2 + 64 * kx : C_WQ2 + 64 * kx + 64] = wq[:, :, 2, kx].T
    # wkv pair: rows c -> kx=0, rows 64+c -> kx=1, per ky; cols [K(64) | V(64)]
    for ky in range(3):
        o = C_WKVP + 128 * ky
        blob16[0:64, o : o + 64] = wk[:, :, ky, 0].T
        blob16[0:64, o + 64 : o + 128] = wv[:, :, ky, 0].T
        blob16[64:128, o : o + 64] = wk[:, :, ky, 1].T
        blob16[64:128, o + 64 : o + 128] = wv[:, :, ky, 1].T
        o = C_WKV2 + 128 * ky
        blob16[64:128, o : o + 64] = wk[:, :, ky, 2].T
        blob16[64:128, o + 64 : o + 128] = wv[:, :, ky, 2].T
    blob16[64:128, C_ID : C_ID + 64] = np.eye(64, dtype=f32)

    # MLP weights
    m1w = np.asarray(mlp_w1, f32)
    m1b = np.asarray(mlp_b1, f32)
    m2w = np.asarray(mlp_w2, f32)
    m2b = np.asarray(mlp_b2, f32)
    rel_cell = cell[0] * np.array([HF, WF], f32)
    b1pp = m1b + m1w[:, 64:66] @ rel_cell
    blob16[0:32, C_W1 : C_W1 + 256] = m1w[:, 0:32].T
    blob16[32:64, C_W1 : C_W1 + 256] = m1w[:, 32:64].T
    blob16[64, C_W1 : C_W1 + 256] = b1pp
    blob16[:, C_W2 : C_W2 + 64] = m2w[:, 0:128].T
    blob16[:, C_W2 + 64 : C_W2 + 128] = m2w[:, 128:256].T

    # position-bias table -> multiplicative exp table with window mask
    w1 = np.asarray(cpb_w1, f32)
    b1 = np.asarray(cpb_b1, f32)
    w2c = np.asarray(cpb_w2, f32)
    dy = (np.linspace(-R, R, 2 * R + 1).astype(f32)) * f32(2.0 / HF)
    delta = np.stack(np.meshgrid(dy, dy, indexing="ij"), -1).reshape(-1, 2)
    pb = np.zeros((HEAD, 2, 2, 7, 7), f32)
    for iy in range(2):
        for jx in range(2):
            sy = f32(-1.0) + (2 * iy + 1) / f32(HQ)
            sx = f32(-1.0) + (2 * jx + 1) / f32(WQ)
            base = np.array([f32(-1.0) + 1.0 / f32(HF), f32(-1.0) + 1.0 / f32(WF)], f32)
            ck = base[None, :] + delta
            rel = (np.array([sy, sx], f32)[None, :] - ck) * np.array([HQ, WQ], f32)
            h = np.maximum(rel @ w1.T + b1, 0.0)
            p = h @ w2c.T
            pb[:, iy, jx] = p.T.reshape(HEAD, 7, 7)

    exptab = np.zeros((BKP, 128), f32)
    for kp in range(BKP):
        xk, py = kp // PROWS, kp % PROWS
        for n in range(128):
            h, r = n // 64, n % 64
            qy, qx = r // 4, r % 4
            dyy = py - (qy // 2 + R)
            dxx = xk - (qx // 2 + R)
            if abs(dyy) <= R and abs(dxx) <= R:
                exptab[kp, n] = np.exp(pb[h, qy % 2, qx % 2, dyy + R, dxx + R])
    blob16[0:BKP, C_EXP : C_EXP + 512] = np.tile(exptab, (1, 4))

    blob32 = np.zeros((128, 2), f32)
    blob32[0:64, 0] = np.asarray(conv_q_b, f32) / np.sqrt(f32(DH))
    blob32[0:64, 1] = np.asarray(conv_k_b, f32)
    blob32[64:128, 1] = np.asarray(conv_v_b, f32)

    # padded global maps
    dpad = np.zeros((64, HQ + 2, WQ + 2), f32)
    dpad[:, 1 : 1 + HQ, 1 : 1 + WQ] = depth_T
    fpad = np.zeros((64, HF + 9, HF + 7), f32)
    fpad[:, 4 : 4 + HF, 3 : 3 + WF] = feat_T
    bidx = np.arange(NBLK)[:, None] * 4 + np.arange(6)[None, :]  # (32, 6)

    in_maps = []
    blob16 = blob16.astype(BF)
    for t in range(N_CORES):
        base = dpad[:, 16 * t : 16 * t + 18, :]          # (64, 18, 130)
        d2a = base[:, 0:17, :][:, :, bidx]               # (64, 17, 32, 6)
        d2b = base[:, 1:18, :][:, :, bidx]
        d2blk = np.concatenate([d2a.transpose(0, 2, 1, 3),
                                d2b.transpose(0, 2, 1, 3)], 0)
        f2a = fpad[:, 8 * t : 8 * t + 16, 0:PW].transpose(0, 2, 1)  # (64, 70, 16)
        f2b = fpad[:, 8 * t : 8 * t + 16, 1 : 1 + PW].transpose(0, 2, 1)
        f2d = np.concatenate([f2a, f2b], 0)

        mrow = np.zeros((PROWS,), f32)
        for py in range(PROWS):
            if 0 <= FROWS * t - R + py < HF:
                mrow[py] = 1.0
        b16 = blob16.copy()
        b16[:, C_MSK : C_MSK + 448] = np.tile(mrow, (128, 32)).astype(BF)

        dres_rows = depth[0, QCORE * t : QCORE * (t + 1), :] + m2b[None, :]
        dres3 = dres_rows[_QPERM].reshape(16, 128, 64).transpose(1, 0, 2)

        m = dict(blob16=np.ascontiguousarray(b16),
                 blob32=blob32,
                 d2blk=np.ascontiguousarray(d2blk.astype(BF)),
                 f2d=np.ascontiguousarray(f2d.astype(BF)),
                 dres3=np.ascontiguousarray(dres3.astype(f32)))
        in_maps.append(m)
    return in_maps


LAST_RESULT = None


def _prep_inputs(inputs):
    keys = dict(inputs)
    keys.pop("shape_y", None)
    keys.pop("shape_x", None)
    return _host_prep(**keys)


def kernel(**inputs):
    global LAST_RESULT
    from concourse.bass_utils import run_bass_kernel_spmd

    if "nc" not in _CACHE:
        _CACHE["nc"] = _build_program()
    nc = _CACHE["nc"]
    in_maps = _prep_inputs(inputs)
    res = run_bass_kernel_spmd(nc, in_maps, core_ids=list(range(N_CORES)))
    LAST_RESULT = res
    parts = []
    for t in range(N_CORES):
        oc3 = np.asarray(res.results[t]["out"]).reshape(128, 16, 64)
        oc = oc3.transpose(1, 0, 2).reshape(QCORE, 64)
        orow = np.empty_like(oc)
        orow[_QPERM] = oc
        parts.append(orow)
    out = np.concatenate(parts, 0)
    return out[None].astype(np.float32)


def _patch_sim_gelu():
    import math
    import concourse.bass_interp as bi
    import concourse.mybir as mb
    if getattr(bi.InstructionExecutor, "_gelu_patched", False):
        return
    orig = bi.InstructionExecutor.visit_InstActivation
    from concourse.bass_interp import Direction

    erf = np.vectorize(math.erf)

    def patched(self, instruction, *, reg_snapshot=None):
        if getattr(instruction, "func", None) == mb.ActivationFunctionType.Gelu:
            instruction.func = mb.ActivationFunctionType.Identity
            try:
                r = orig(self, instruction, reg_snapshot=reg_snapshot)
            finally:
                instruction.func = mb.ActivationFunctionType.Gelu
            ov = self.view_ap(instruction.outs[0], Direction.WRITE, instruction,
                              reg_snapshot=reg_snapshot)
            x = np.asarray(ov[:], dtype=np.float64)
            ov[:] = (0.5 * x * (1.0 + erf(x / np.sqrt(2.0)))).astype(np.float32)
            return r
        return orig(self, instruction, reg_snapshot=reg_snapshot)

    bi.InstructionExecutor.visit_InstActivation = patched
    bi.InstructionExecutor._gelu_patched = True


def simulate_core(core=0, inputs=None, dbg=False):
    """CoreSim single-core check helper (dev only)."""
    from concourse.bass_interp import CoreSim

    _patch_sim_gelu()

    if "nc" not in _CACHE:
        _CACHE["nc"] = _build_program()
    nc = _CACHE["nc"]
    in_maps = _prep_inputs(inputs)
    sim = CoreSim(nc, trace=False)
    for k, v in in_maps[core].items():
        sim.tensor(k)[:] = v
    sim.simulate(check_with_hw=False)
    oc3 = np.array(sim.tensor("out")).reshape(128, 16, 64)
    return oc3.transpose(1, 0, 2).reshape(QCORE, 64)


# revision 14
# speedup vs baseline: 2.4098x; 1.2039x over previous
"""Trainium2 Bass kernel for nn_CrossLITFusion (sparse window attention fusion).

Self-contained: hardcodes all shapes. Shards the query-pixel axis across the
8 NeuronCores (16 query rows / 8 feature rows + 3-row halo per core).

v2: host-side layout prep (block-major depth, x-major feat), transposed MLP1
(no DMA transposes), in-matmul softmax denominators, batched DMAs.
"""
import sys

sys.path.insert(0, "/opt/trn_rl_repo")

from contextlib import ExitStack

import ml_dtypes
import numpy as np

import concourse.bass as bass
import concourse.bacc as bacc
import concourse.mybir as mybir
import concourse.tile as tile

BF = ml_dtypes.bfloat16
dt = mybir.dt
AF = mybir.ActivationFunctionType
ALU = mybir.AluOpType

# Problem constants
DIM = 64
HEAD = 2
DH = 32
R = 3
HF, WF = 64, 64
HQ, WQ = 128, 128
HID = 256
N_CORES = 8

QROWS = 16             # query rows per core
FROWS = 8              # feature rows owned
PROWS = FROWS + 2 * R  # 14 padded feature rows (halo)
PW = WF + 2 * R        # 70 padded feature cols
NBLK = WF // 2         # 32 col-blocks (4 query cols each)
BKP = 8 * PROWS        # 112 kpix per block window
NSB = 8                # superblocks (4 blocks each)
QCORE = QROWS * WQ     # 2048 queries per core

# blob16 column offsets
C_WQP = 0       # [128, 3, 64]
C_WQ2 = 192     # [64:128, 3, 64]
C_WKVP = 384    # [128, 3, 128]
C_WKV2 = 768    # [64:128, 3, 128]
C_ID = 1152     # [64:128, 64]
C_W1 = 1216     # [0:65, 256]
C_W2 = 1472     # [128, 2, 64]
C_MSK = 1600    # [128, 32*14]
C_EXP = 2048    # [0:112, 512]
NB16 = 2560

_CACHE = {}

# attention pipeline orders queries block-major: qa = 64*b + 4*qy + qxl
# (b = feat col-block = qx//4). _QPERM[qa] = row-major index 128*qy + 4*b + qxl.
_QA = np.arange(QCORE)
_QPERM = 128 * ((_QA % 64) // 4) + 4 * (_QA // 64) + (_QA % 4)


def _build_program():
    nc = bacc.Bacc("TRN2", target_bir_lowering=False, debug=False)

    # ---- DRAM I/O ----
    blob16_d = nc.dram_tensor("blob16", [128, NB16], dt.bfloat16, kind="ExternalInput").ap()
    blob32_d = nc.dram_tensor("blob32", [128, 2], dt.float32, kind="ExternalInput").ap()
    d2_d = nc.dram_tensor("d2blk", [128, NBLK, 17, 6], dt.bfloat16, kind="ExternalInput").ap()
    f2_d = nc.dram_tensor("f2d", [128, PW, 16], dt.bfloat16, kind="ExternalInput").ap()
    dres_d = nc.dram_tensor("dres3", [64, QCORE], dt.float32, kind="ExternalInput").ap()
    out_d = nc.dram_tensor("out", [64, QCORE], dt.float32, kind="ExternalOutput").ap()

    with tile.TileContext(nc) as tc, ExitStack() as ctx:
        const = ctx.enter_context(tc.tile_pool(name="const", bufs=1))
        maps = ctx.enter_context(tc.tile_pool(name="maps", bufs=1))

        # ---- SBUF tiles ----
        cb16 = const.tile([128, NB16], dt.bfloat16)
        cb32 = const.tile([128, 2], dt.float32)
        d2 = maps.tile([128, NBLK, 17, 6], dt.bfloat16)
        f2 = maps.tile([128, PW, 16], dt.bfloat16)
        dresT = maps.tile([64, QCORE], dt.float32)
        qmap = maps.tile([64, NBLK, 64], dt.bfloat16)
        q2 = maps.tile([64, NBLK, 128], dt.bfloat16)
        kv = maps.tile([128, PW, PROWS], dt.bfloat16)
        vt = maps.tile([BKP, NBLK, 96], dt.bfloat16)
        attn = maps.tile([65, QCORE], dt.bfloat16)
        hg = maps.tile([128, 2, QCORE], dt.bfloat16)
        oT = maps.tile([64, QCORE], dt.float32)

        # ---- input DMAs: sync queue ----
        nc.sync.dma_start(cb16[:], blob16_d)
        nc.sync.dma_start(cb32[:], blob32_d)
        nc.sync.dma_start(d2[:], d2_d)
        nc.sync.dma_start(dresT[:], dres_d)
        # scalar queue
        nc.scalar.dma_start(f2[:], f2_d)

        # ---- const views ----
        wqp = cb16[:, C_WQP : C_WQP + 192].rearrange("c (k m) -> c k m", k=3)
        wq2 = cb16[64:128, C_WQ2 : C_WQ2 + 192].rearrange("c (k m) -> c k m", k=3)
        wkvp = cb16[:, C_WKVP : C_WKVP + 384].rearrange("c (k m) -> c k m", k=3)
        wkv2 = cb16[64:128, C_WKV2 : C_WKV2 + 384].rearrange("c (k m) -> c k m", k=3)
        ident = cb16[64:128, C_ID : C_ID + 64]
        w1a = cb16[0:65, C_W1 : C_W1 + 256]
        w2 = cb16[:, C_W2 : C_W2 + 128].rearrange("c (g m) -> c g m", g=2)
        msk = cb16[:, C_MSK : C_MSK + 448].rearrange("c (x y) -> c x y", x=32)
        expt = cb16[0:112, C_EXP : C_EXP + 512]
        bq = cb32[0:64, 0:1]
        bkv = cb32[:, 1:2]

        # ---- memsets (off critical path) ----
        nc.gpsimd.memset(q2[0:32, :, 64:128], 0.0)
        nc.gpsimd.memset(q2[32:64, :, 0:64], 0.0)
        nc.vector.memset(kv[:, 0:R, :], 0.0)
        nc.vector.memset(kv[:, R + WF : PW, :], 0.0)
        nc.gpsimd.memset(vt[:, :, 64:96], 1.0)
        nc.vector.memset(attn[64:65, :], 1.0)

        # ---- conv_k + conv_v fused -> KV x-major (128, 70, 14) ----
        # kv[c, x, y]: K channels rows 0-63, V rows 64-127
        with tc.tile_pool(name="kv_ps", bufs=2, space="PSUM") as kv_ps:
            for xc in range(2):
                x0 = R + 32 * xc
                acc = kv_ps.tile([128, 32, PROWS], dt.float32, tag="kv")
                for ky in range(3):
                    nc.tensor.matmul(acc[:], wkvp[:, ky, :],
                                     f2[:, x0 - 1 : x0 + 31, ky : ky + PROWS],
                                     start=(ky == 0), stop=False)
                for ky in range(3):
                    nc.tensor.matmul(acc[:], wkv2[:, ky, :],
                                     f2[64:128, x0 : x0 + 32, ky : ky + PROWS],
                                     start=False, stop=(ky == 2))
                # (conv + bias) * row-validity mask
                nc.vector.scalar_tensor_tensor(
                    kv[0:64, x0 : x0 + 32, :], acc[0:64], bkv[0:64],
                    msk[0:64], op0=ALU.add, op1=ALU.mult)
                nc.vector.scalar_tensor_tensor(
                    kv[64:128, x0 : x0 + 32, :], acc[64:128], bkv[64:128],
                    msk[64:128], op0=ALU.add, op1=ALU.mult)

        # ---- V_T tiles (112, 96) per block: [Vh0(32) | Vh1(32) | ones(32)] ----
        with tc.tile_pool(name="vt_ps", bufs=3, space="PSUM") as vt_ps:
            for b in range(NBLK):
                tp = vt_ps.tile([BKP, 64], dt.bfloat16, tag="vt")
                nc.tensor.transpose(tp[:], kv[64:128, 2 * b : 2 * b + 8, :], ident)
                nc.scalar.activation(vt[:, b, 0:64], tp[:], AF.Identity)

        # ---- phase 1: conv_q chunks + attention ----
        qsc = float(1.0 / np.sqrt(DH))
        with tc.tile_pool(name="cq_ps", bufs=2, space="PSUM") as cq_ps, \
             tc.tile_pool(name="sc_ps", bufs=3, space="PSUM") as sc_ps, \
             tc.tile_pool(name="p_pool", bufs=2) as p_pool:

            def conv_q_chunk(c):
                b0 = 8 * c
                acc = cq_ps.tile([64, 512], dt.float32, tag="cq")
                accv = acc[:].rearrange("c (b y x) -> c b y x", b=8, y=16)
                for kx in range(3):
                    nc.tensor.matmul(accv, wqp[:, kx, :],
                                     d2[:, b0 : b0 + 8, 0:16, kx : kx + 4],
                                     start=(kx == 0), stop=False)
                for kx in range(3):
                    nc.tensor.matmul(accv, wq2[:, kx, :],
                                     d2[64:128, b0 : b0 + 8, 1:17, kx : kx + 4],
                                     start=False, stop=(kx == 2))
                nc.vector.tensor_scalar(qmap[:, b0 : b0 + 8, :], accv,
                                        qsc, bq, op0=ALU.mult, op1=ALU.add)
                nc.sync.dma_start(q2[0:32, b0 : b0 + 8, 0:64],
                                  qmap[0:32, b0 : b0 + 8, :])
                nc.scalar.dma_start(q2[32:64, b0 : b0 + 8, 64:128],
                                    qmap[32:64, b0 : b0 + 8, :])

            def attention_sb(sb):
                qkp = sc_ps.tile([128, 512], dt.float32, tag="sc")
                for j in range(4):
                    b = 4 * sb + j
                    nc.tensor.matmul(qkp[0:112, 128 * j : 128 * j + 128],
                                     kv[0:64, 2 * b : 2 * b + 8, :], q2[:, b, :],
                                     start=True, stop=True)
                pex = p_pool.tile([BKP, 512], dt.bfloat16, tag="pex")
                nc.scalar.activation(pex[:], qkp[0:112, :], AF.Exp)
                pw = p_pool.tile([BKP, 512], dt.bfloat16, tag="pw")
                nc.vector.tensor_mul(pw[:], pex[:], expt)
                avp = sc_ps.tile([128, 4, 128], dt.float32, tag="sc")
                for j in range(4):
                    b = 4 * sb + j
                    nc.tensor.matmul(avp[0:96, j, :], vt[:, b, :],
                                     pw[:, 128 * j : 128 * j + 128],
                                     start=True, stop=True)
                a0 = attn[0:32, 256 * sb : 256 * sb + 256].rearrange(
                    "p (j c) -> p j c", j=4)
                a1 = attn[32:64, 256 * sb : 256 * sb + 256].rearrange(
                    "p (j c) -> p j c", j=4)
                den = p_pool.tile([64, 4, 64], dt.float32, tag="den")
                nc.vector.tensor_copy(den[0:32], avp[64:96, :, 0:64])
                nc.vector.tensor_copy(den[32:64], avp[64:96, :, 64:128])
                rcp = p_pool.tile([64, 4, 64], dt.float32, tag="rcp")
                nc.vector.reciprocal_approx_fast(rcp[:], den[:])
                nc.vector.tensor_mul(a0, avp[0:32, :, 0:64], rcp[0:32])
                nc.vector.tensor_mul(a1, avp[32:64, :, 64:128], rcp[32:64])

            for c in range(4):
                conv_q_chunk(c)
                attention_sb(2 * c)
                attention_sb(2 * c + 1)

        # ---- phase 2: MLP (batched activations: one Gelu table load) ----
        with tc.tile_pool(name="m1_ps", bufs=2, space="PSUM") as m1_ps, \
             tc.tile_pool(name="m2_ps", bufs=2, space="PSUM") as m2_ps:
            for c in range(4):
                for g in range(2):
                    m1 = m1_ps.tile([128, 512], dt.float32, tag="m1")
                    nc.tensor.matmul(m1[:], w1a[:, 128 * g : 128 * g + 128],
                                     attn[:, 512 * c : 512 * c + 512],
                                     start=True, stop=True)
                    nc.scalar.activation(hg[:, g, 512 * c : 512 * c + 512],
                                         m1[:], AF.Gelu)
                m2 = m2_ps.tile([64, 512], dt.float32, tag="m2")
                nc.tensor.matmul(m2[:], w2[:, 0, :], hg[:, 0, 512 * c : 512 * c + 512],
                                 start=True, stop=False)
                nc.tensor.matmul(m2[:], w2[:, 1, :], hg[:, 1, 512 * c : 512 * c + 512],
                                 start=False, stop=True)
                nc.vector.tensor_add(oT[:, 512 * c : 512 * c + 512], m2[:],
                                     dresT[:, 512 * c : 512 * c + 512])

        nc.sync.dma_start(out_d, oT[:])

    nc.compile()
    return nc


def _host_prep(depth, x, cell, conv_q_w, conv_q_b, conv_k_w, conv_k_b,
               conv_v_w, conv_v_b, cpb_w1, cpb_b1, cpb_w2,
               mlp_w1, mlp_b1, mlp_w2, mlp_b2):
    """Build the 8 per-core input maps."""
    f32 = np.float32
    depth = np.asarray(depth, f32)
    x = np.asarray(x, f32)
    cell = np.asarray(cell, f32)

    depth_T = np.ascontiguousarray(depth[0].T).reshape(64, HQ, WQ)
    feat_T = np.ascontiguousarray(x[0].T).reshape(64, HF, WF)

    wq = np.asarray(conv_q_w, f32)
    wk = np.asarray(conv_k_w, f32)
    wv = np.asarray(conv_v_w, f32)

    blob16 = np.zeros((128, NB16), f32)
    # wq pair: rows c -> ky=0, rows 64+c -> ky=1, per kx
    for kx in range(3):
        blob16[0:64, C_WQP + 64 * kx : C_WQP + 64 * kx + 64] = wq[:, :, 0, kx].T
        blob16[64:128, C_WQP + 64 * kx : C_WQP + 64 * kx + 64] = wq[:, :, 1, kx].T
        blob16[64:128, C_WQ2 + 64 * kx : C_WQ2 + 64 * kx + 64] = wq[:, :, 2, kx].T
    # wkv pair: rows c -> kx=0, rows 64+c -> kx=1, per ky; cols [K(64) | V(64)]
    for ky in range(3):
        o = C_WKVP + 128 * ky
        blob16[0:64, o : o + 64] = wk[:, :, ky, 0].T
        blob16[0:64, o + 64 : o + 128] = wv[:, :, ky, 0].T
        blob16[64:128, o : o + 64] = wk[:, :, ky, 1].T
        blob16[64:128, o + 64 : o + 128] = wv[:, :, ky, 1].T
        o = C_WKV2 + 128 * ky
        blob16[64:128, o : o + 64] = wk[:, :, ky, 2].T
        blob16[64:128, o + 64 : o + 128] = wv[:, :, ky, 2].T
    blob16[64:128, C_ID : C_ID + 64] = np.eye(64, dtype=f32)

    # MLP weights
    m1w = np.asarray(mlp_w1, f32)
    m1b = np.asarray(mlp_b1, f32)
    m2w = np.asarray(mlp_w2, f32)
    m2b = np.asarray(mlp_b2, f32)
    rel_cell = cell[0] * np.array([HF, WF], f32)
    b1pp = m1b + m1w[:, 64:66] @ rel_cell
    blob16[0:32, C_W1 : C_W1 + 256] = m1w[:, 0:32].T
    blob16[32:64, C_W1 : C_W1 + 256] = m1w[:, 32:64].T
    blob16[64, C_W1 : C_W1 + 256] = b1pp
    blob16[:, C_W2 : C_W2 + 64] = m2w[:, 0:128].T
    blob16[:, C_W2 + 64 : C_W2 + 128] = m2w[:, 128:256].T

    # position-bias table -> multiplicative exp table with window mask
    w1 = np.asarray(cpb_w1, f32)
    b1 = np.asarray(cpb_b1, f32)
    w2c = np.asarray(cpb_w2, f32)
    dy = (np.linspace(-R, R, 2 * R + 1).astype(f32)) * f32(2.0 / HF)
    delta = np.stack(np.meshgrid(dy, dy, indexing="ij"), -1).reshape(-1, 2)
    pb = np.zeros((HEAD, 2, 2, 7, 7), f32)
    for iy in range(2):
        for jx in range(2):
            sy = f32(-1.0) + (2 * iy + 1) / f32(HQ)
            sx = f32(-1.0) + (2 * jx + 1) / f32(WQ)
            base = np.array([f32(-1.0) + 1.0 / f32(HF), f32(-1.0) + 1.0 / f32(WF)], f32)
            ck = base[None, :] + delta
            rel = (np.array([sy, sx], f32)[None, :] - ck) * np.array([HQ, WQ], f32)
            h = np.maximum(rel @ w1.T + b1, 0.0)
            p = h @ w2c.T
            pb[:, iy, jx] = p.T.reshape(HEAD, 7, 7)

    exptab = np.zeros((BKP, 128), f32)
    for kp in range(BKP):
        xk, py = kp // PROWS, kp % PROWS
        for n in range(128):
            h, r = n // 64, n % 64
            qy, qx = r // 4, r % 4
            dyy = py - (qy // 2 + R)
            dxx = xk - (qx // 2 + R)
            if abs(dyy) <= R and abs(dxx) <= R:
                exptab[kp, n] = np.exp(pb[h, qy % 2, qx % 2, dyy + R, dxx + R])
    blob16[0:BKP, C_EXP : C_EXP + 512] = np.tile(exptab, (1, 4))

    blob32 = np.zeros((128, 2), f32)
    blob32[0:64, 0] = np.asarray(conv_q_b, f32) / np.sqrt(f32(DH))
    blob32[0:64, 1] = np.asarray(conv_k_b, f32)
    blob32[64:128, 1] = np.asarray(conv_v_b, f32)

    # padded global maps
    dpad = np.zeros((64, HQ + 2, WQ + 2), f32)
    dpad[:, 1 : 1 + HQ, 1 : 1 + WQ] = depth_T
    fpad = np.zeros((64, HF + 9, HF + 7), f32)
    fpad[:, 4 : 4 + HF, 3 : 3 + WF] = feat_T
    bidx = np.arange(NBLK)[:, None] * 4 + np.arange(6)[None, :]  # (32, 6)

    in_maps = []
    blob16 = blob16.astype(BF)
    for t in range(N_CORES):
        base = dpad[:, 16 * t : 16 * t + 18, :]          # (64, 18, 130)
        d2a = base[:, 0:17, :][:, :, bidx]               # (64, 17, 32, 6)
        d2b = base[:, 1:18, :][:, :, bidx]
        d2blk = np.concatenate([d2a.transpose(0, 2, 1, 3),
                                d2b.transpose(0, 2, 1, 3)], 0)
        f2a = fpad[:, 8 * t : 8 * t + 16, 0:PW].transpose(0, 2, 1)  # (64, 70, 16)
        f2b = fpad[:, 8 * t : 8 * t + 16, 1 : 1 + PW].transpose(0, 2, 1)
        f2d = np.concatenate([f2a, f2b], 0)

        mrow = np.zeros((PROWS,), f32)
        for py in range(PROWS):
            if 0 <= FROWS * t - R + py < HF:
                mrow[py] = 1.0
        b16 = blob16.copy()
        b16[:, C_MSK : C_MSK + 448] = np.tile(mrow, (128, 32)).astype(BF)

        dres_rows = depth[0, QCORE * t : QCORE * (t + 1), :] + m2b[None, :]
        dres3 = dres_rows[_QPERM].T

        m = dict(blob16=np.ascontiguousarray(b16),
                 blob32=blob32,
                 d2blk=np.ascontiguousarray(d2blk.astype(BF)),
                 f2d=np.ascontiguousarray(f2d.astype(BF)),
                 dres3=np.ascontiguousarray(dres3.astype(f32)))
        in_maps.append(m)
    return in_maps


LAST_RESULT = None


def _prep_inputs(inputs):
    keys = dict(inputs)
    keys.pop("shape_y", None)
    keys.pop("shape_x", None)
    return _host_prep(**keys)


def kernel(**inputs):
    global LAST_RESULT
    from concourse.bass_utils import run_bass_kernel_spmd

    if "nc" not in _CACHE:
        _CACHE["nc"] = _build_program()
    nc = _CACHE["nc"]
    in_maps = _prep_inputs(inputs)
    res = run_bass_kernel_spmd(nc, in_maps, core_ids=list(range(N_CORES)))
    LAST_RESULT = res
    parts = []
    for t in range(N_CORES):
        oc = np.ascontiguousarray(np.asarray(res.results[t]["out"]).reshape(64, QCORE).T)
        orow = np.empty_like(oc)
        orow[_QPERM] = oc
        parts.append(orow)
    out = np.concatenate(parts, 0)
    return out[None].astype(np.float32)


def _patch_sim_gelu():
    import math
    import concourse.bass_interp as bi
    import concourse.mybir as mb
    if getattr(bi.InstructionExecutor, "_gelu_patched", False):
        return
    orig = bi.InstructionExecutor.visit_InstActivation
    from concourse.bass_interp import Direction

    erf = np.vectorize(math.erf)

    def patched(self, instruction, *, reg_snapshot=None):
        if getattr(instruction, "func", None) == mb.ActivationFunctionType.Gelu:
            instruction.func = mb.ActivationFunctionType.Identity
            try:
                r = orig(self, instruction, reg_snapshot=reg_snapshot)
            finally:
                instruction.func = mb.ActivationFunctionType.Gelu
            ov = self.view_ap(instruction.outs[0], Direction.WRITE, instruction,
                              reg_snapshot=reg_snapshot)
            x = np.asarray(ov[:], dtype=np.float64)
            ov[:] = (0.5 * x * (1.0 + erf(x / np.sqrt(2.0)))).astype(np.float32)
            return r
        return orig(self, instruction, reg_snapshot=reg_snapshot)

    bi.InstructionExecutor.visit_InstActivation = patched
    bi.InstructionExecutor._gelu_patched = True


def simulate_core(core=0, inputs=None, dbg=False):
    """CoreSim single-core check helper (dev only)."""
    from concourse.bass_interp import CoreSim

    _patch_sim_gelu()

    if "nc" not in _CACHE:
        _CACHE["nc"] = _build_program()
    nc = _CACHE["nc"]
    in_maps = _prep_inputs(inputs)
    sim = CoreSim(nc, trace=False)
    for k, v in in_maps[core].items():
        sim.tensor(k)[:] = v
    sim.simulate(check_with_hw=False)
    return np.ascontiguousarray(np.array(sim.tensor("out")).reshape(64, QCORE).T)


# revision 20
# speedup vs baseline: 2.4745x; 1.0268x over previous
"""Trainium2 Bass kernel for nn_CrossLITFusion (sparse window attention fusion).

Self-contained: hardcodes all shapes. Shards the query-pixel axis across the
8 NeuronCores (16 query rows / 8 feature rows + 3-row halo per core).

v2: host-side layout prep (block-major depth, x-major feat), transposed MLP1
(no DMA transposes), in-matmul softmax denominators, batched DMAs.
"""
import sys

sys.path.insert(0, "/opt/trn_rl_repo")

from contextlib import ExitStack

import ml_dtypes
import numpy as np

import concourse.bass as bass
import concourse.bacc as bacc
import concourse.mybir as mybir
import concourse.tile as tile

BF = ml_dtypes.bfloat16
dt = mybir.dt
AF = mybir.ActivationFunctionType
ALU = mybir.AluOpType

# Problem constants
DIM = 64
HEAD = 2
DH = 32
R = 3
HF, WF = 64, 64
HQ, WQ = 128, 128
HID = 256
N_CORES = 8

QROWS = 16             # query rows per core
FROWS = 8              # feature rows owned
PROWS = FROWS + 2 * R  # 14 padded feature rows (halo)
PW = WF + 2 * R        # 70 padded feature cols
NBLK = WF // 2         # 32 col-blocks (4 query cols each)
BKP = 8 * PROWS        # 112 kpix per block window
NSB = 8                # superblocks (4 blocks each)
QCORE = QROWS * WQ     # 2048 queries per core

# blob16 column offsets
C_WQP = 0       # [128, 3, 64]
C_WQ2 = 192     # [64:128, 3, 64]
C_WKVP = 384    # [128, 3, 128]
C_WKV2 = 768    # [64:128, 3, 128]
C_ID = 1152     # [64:128, 64]
C_W1 = 1216     # [0:65, 256]
C_W2 = 1472     # [128, 2, 64]
C_MSK = 1600    # [128, 32*14]
C_EXP = 2048    # [0:112, 512]
NB16 = 2560

_CACHE = {}

# attention pipeline orders queries block-major: qa = 64*b + 4*qy + qxl
# (b = feat col-block = qx//4). _QPERM[qa] = row-major index 128*qy + 4*b + qxl.
_QA = np.arange(QCORE)
_QPERM = 128 * ((_QA % 64) // 4) + 4 * (_QA // 64) + (_QA % 4)


def _build_program():
    nc = bacc.Bacc("TRN2", target_bir_lowering=False, debug=False)

    # ---- DRAM I/O ----
    blob16_d = nc.dram_tensor("blob16", [128, NB16], dt.bfloat16, kind="ExternalInput").ap()
    blob32_d = nc.dram_tensor("blob32", [128, 2], dt.float32, kind="ExternalInput").ap()
    d2_d = nc.dram_tensor("d2blk", [128, NBLK, 17, 6], dt.bfloat16, kind="ExternalInput").ap()
    f2_d = nc.dram_tensor("f2d", [128, PW, 16], dt.bfloat16, kind="ExternalInput").ap()
    dres_d = nc.dram_tensor("dres3", [64, QCORE], dt.float32, kind="ExternalInput").ap()
    out_d = nc.dram_tensor("out", [64, QCORE], dt.float32, kind="ExternalOutput").ap()

    with tile.TileContext(nc) as tc, ExitStack() as ctx:
        const = ctx.enter_context(tc.tile_pool(name="const", bufs=1))
        maps = ctx.enter_context(tc.tile_pool(name="maps", bufs=1))

        # ---- SBUF tiles ----
        cb16 = const.tile([128, NB16], dt.bfloat16)
        cb32 = const.tile([128, 2], dt.float32)
        d2 = maps.tile([128, NBLK, 17, 6], dt.bfloat16)
        f2 = maps.tile([128, PW, 16], dt.bfloat16)
        dresT = maps.tile([64, QCORE], dt.float32)
        qmap = maps.tile([64, NBLK, 64], dt.bfloat16)
        q2 = maps.tile([64, NBLK, 128], dt.bfloat16)
        kv = maps.tile([128, PW, PROWS], dt.bfloat16)
        vt = maps.tile([BKP, NBLK, 96], dt.bfloat16)
        attn = maps.tile([65, QCORE], dt.bfloat16)
        hg = maps.tile([128, 2, QCORE], dt.bfloat16)
        oT = maps.tile([64, QCORE], dt.float32)

        # ---- input DMAs: spread across queues; cb16/f2 gate the first convs ----
        nc.sync.dma_start(cb16[:], blob16_d)
        nc.sync.dma_start(cb32[:], blob32_d)
        nc.scalar.dma_start(f2[:], f2_d)
        nc.sync.dma_start(d2[:], d2_d)
        nc.gpsimd.dma_start(dresT[:], dres_d)

        # ---- const views ----
        wqp = cb16[:, C_WQP : C_WQP + 192].rearrange("c (k m) -> c k m", k=3)
        wq2 = cb16[64:128, C_WQ2 : C_WQ2 + 192].rearrange("c (k m) -> c k m", k=3)
        wkvp = cb16[:, C_WKVP : C_WKVP + 384].rearrange("c (k m) -> c k m", k=3)
        wkv2 = cb16[64:128, C_WKV2 : C_WKV2 + 384].rearrange("c (k m) -> c k m", k=3)
        ident = cb16[64:128, C_ID : C_ID + 64]
        w1a = cb16[0:65, C_W1 : C_W1 + 256]
        w2 = cb16[:, C_W2 : C_W2 + 128].rearrange("c (g m) -> c g m", g=2)
        msk = cb16[:, C_MSK : C_MSK + 448].rearrange("c (x y) -> c x y", x=32)
        expt = cb16[0:112, C_EXP : C_EXP + 512]
        bq = cb32[0:64, 0:1]
        bkv = cb32[:, 1:2]

        # ---- memsets (off critical path) ----
        nc.gpsimd.memset(q2[0:32, :, 64:128], 0.0)
        nc.gpsimd.memset(q2[32:64, :, 0:64], 0.0)
        nc.vector.memset(kv[:, 0:R, :], 0.0)
        nc.vector.memset(kv[:, R + WF : PW, :], 0.0)
        nc.gpsimd.memset(vt[:, :, 64:96], 1.0)
        nc.vector.memset(attn[64:65, :], 1.0)

        # ---- conv_k + conv_v fused -> KV x-major (128, 70, 14) ----
        # kv[c, x, y]: K channels rows 0-63, V rows 64-127
        with tc.tile_pool(name="kv_ps", bufs=2, space="PSUM") as kv_ps:
            for xc in range(2):
                x0 = R + 32 * xc
                acc = kv_ps.tile([128, 32, PROWS], dt.float32, tag="kv")
                for ky in range(3):
                    nc.tensor.matmul(acc[:], wkvp[:, ky, :],
                                     f2[:, x0 - 1 : x0 + 31, ky : ky + PROWS],
                                     start=(ky == 0), stop=False)
                for ky in range(3):
                    nc.tensor.matmul(acc[:], wkv2[:, ky, :],
                                     f2[64:128, x0 : x0 + 32, ky : ky + PROWS],
                                     start=False, stop=(ky == 2))
                # (conv + bias) * row-validity mask
                nc.vector.scalar_tensor_tensor(
                    kv[0:64, x0 : x0 + 32, :], acc[0:64], bkv[0:64],
                    msk[0:64], op0=ALU.add, op1=ALU.mult)
                nc.vector.scalar_tensor_tensor(
                    kv[64:128, x0 : x0 + 32, :], acc[64:128], bkv[64:128],
                    msk[64:128], op0=ALU.add, op1=ALU.mult)

        # ---- V_T tiles (112, 96) per block: [Vh0(32) | Vh1(32) | ones(32)] ----
        with tc.tile_pool(name="vt_ps", bufs=3, space="PSUM") as vt_ps:
            for b in range(NBLK):
                tp = vt_ps.tile([BKP, 64], dt.bfloat16, tag="vt")
                nc.tensor.transpose(tp[:], kv[64:128, 2 * b : 2 * b + 8, :], ident)
                nc.scalar.copy(vt[:, b, 0:64], tp[:])

        # ---- phase 1: conv_q chunks + attention ----
        qsc = float(1.0 / np.sqrt(DH))
        with tc.tile_pool(name="cq_ps", bufs=2, space="PSUM") as cq_ps, \
             tc.tile_pool(name="sc_ps", bufs=3, space="PSUM") as sc_ps, \
             tc.tile_pool(name="p_pool", bufs=2) as p_pool:

            def conv_q_chunk(c):
                b0 = 8 * c
                acc = cq_ps.tile([64, 512], dt.float32, tag="cq")
                accv = acc[:].rearrange("c (b y x) -> c b y x", b=8, y=16)
                for kx in range(3):
                    nc.tensor.matmul(accv, wqp[:, kx, :],
                                     d2[:, b0 : b0 + 8, 0:16, kx : kx + 4],
                                     start=(kx == 0), stop=False)
                for kx in range(3):
                    nc.tensor.matmul(accv, wq2[:, kx, :],
                                     d2[64:128, b0 : b0 + 8, 1:17, kx : kx + 4],
                                     start=False, stop=(kx == 2))
                nc.vector.tensor_scalar(qmap[:, b0 : b0 + 8, :], accv,
                                        qsc, bq, op0=ALU.mult, op1=ALU.add)
                nc.sync.dma_start(q2[0:32, b0 : b0 + 8, 0:64],
                                  qmap[0:32, b0 : b0 + 8, :])
                nc.gpsimd.dma_start(q2[32:64, b0 : b0 + 8, 64:128],
                                    qmap[32:64, b0 : b0 + 8, :])

            def attention_sb(sb):
                qkp = sc_ps.tile([128, 512], dt.float32, tag="sc")
                for j in range(4):
                    b = 4 * sb + j
                    nc.tensor.matmul(qkp[0:112, 128 * j : 128 * j + 128],
                                     kv[0:64, 2 * b : 2 * b + 8, :], q2[:, b, :],
                                     start=True, stop=True)
                pex = p_pool.tile([BKP, 512], dt.bfloat16, tag="pex")
                nc.scalar.activation(pex[:], qkp[0:112, :], AF.Exp)
                pw = p_pool.tile([BKP, 512], dt.bfloat16, tag="pw")
                nc.vector.tensor_mul(pw[:], pex[:], expt)
                avp = sc_ps.tile([128, 4, 128], dt.float32, tag="sc")
                for j in range(4):
                    b = 4 * sb + j
                    nc.tensor.matmul(avp[0:96, j, :], vt[:, b, :],
                                     pw[:, 128 * j : 128 * j + 128],
                                     start=True, stop=True)
                a0 = attn[0:32, 256 * sb : 256 * sb + 256].rearrange(
                    "p (j c) -> p j c", j=4)
                a1 = attn[32:64, 256 * sb : 256 * sb + 256].rearrange(
                    "p (j c) -> p j c", j=4)
                den = p_pool.tile([64, 4, 64], dt.float32, tag="den")
                nc.scalar.copy(den[0:32], avp[64:96, :, 0:64])
                nc.scalar.copy(den[32:64], avp[64:96, :, 64:128])
                rcp = p_pool.tile([64, 4, 64], dt.float32, tag="rcp")
                nc.vector.reciprocal_approx_fast(rcp[:], den[:])
                nc.vector.tensor_mul(a0, avp[0:32, :, 0:64], rcp[0:32])
                nc.vector.tensor_mul(a1, avp[32:64, :, 64:128], rcp[32:64])

            for c in range(4):
                conv_q_chunk(c)
                attention_sb(2 * c)
                attention_sb(2 * c + 1)

        # ---- phase 2: MLP; wait_until batches the Gelus after the Exps so the
        # scalar engine loads each activation table once ----
        with tc.tile_pool(name="m1_ps", bufs=2, space="PSUM") as m1_ps, \
             tc.tile_pool(name="m2_ps", bufs=2, space="PSUM") as m2_ps, \
             tc.tile_wait_until(0.046):
            for c in range(4):
                for g in range(2):
                    m1 = m1_ps.tile([128, 512], dt.float32, tag="m1")
                    nc.tensor.matmul(m1[:], w1a[:, 128 * g : 128 * g + 128],
                                     attn[:, 512 * c : 512 * c + 512],
                                     start=True, stop=True)
                    nc.scalar.activation(hg[:, g, 512 * c : 512 * c + 512],
                                         m1[:], AF.Gelu)
                m2 = m2_ps.tile([64, 512], dt.float32, tag="m2")
                nc.tensor.matmul(m2[:], w2[:, 0, :], hg[:, 0, 512 * c : 512 * c + 512],
                                 start=True, stop=False)
                nc.tensor.matmul(m2[:], w2[:, 1, :], hg[:, 1, 512 * c : 512 * c + 512],
                                 start=False, stop=True)
                nc.vector.tensor_add(oT[:, 512 * c : 512 * c + 512], m2[:],
                                     dresT[:, 512 * c : 512 * c + 512])

        nc.sync.dma_start(out_d, oT[:])

    nc.compile()
    return nc


def _host_prep(depth, x, cell, conv_q_w, conv_q_b, conv_k_w, conv_k_b,
               conv_v_w, conv_v_b, cpb_w1, cpb_b1, cpb_w2,
               mlp_w1, mlp_b1, mlp_w2, mlp_b2):
    """Build the 8 per-core input maps."""
    f32 = np.float32
    depth = np.asarray(depth, f32)
    x = np.asarray(x, f32)
    cell = np.asarray(cell, f32)

    depth_T = np.ascontiguousarray(depth[0].T).reshape(64, HQ, WQ)
    feat_T = np.ascontiguousarray(x[0].T).reshape(64, HF, WF)

    wq = np.asarray(conv_q_w, f32)
    wk = np.asarray(conv_k_w, f32)
    wv = np.asarray(conv_v_w, f32)

    blob16 = np.zeros((128, NB16), f32)
    # wq pair: rows c -> ky=0, rows 64+c -> ky=1, per kx
    for kx in range(3):
        blob16[0:64, C_WQP + 64 * kx : C_WQP + 64 * kx + 64] = wq[:, :, 0, kx].T
        blob16[64:128, C_WQP + 64 * kx : C_WQP + 64 * kx + 64] = wq[:, :, 1, kx].T
        blob16[64:128, C_WQ2 + 64 * kx : C_WQ2 + 64 * kx + 64] = wq[:, :, 2, kx].T
    # wkv pair: rows c -> kx=0, rows 64+c -> kx=1, per ky; cols [K(64) | V(64)]
    for ky in range(3):
        o = C_WKVP + 128 * ky
        blob16[0:64, o : o + 64] = wk[:, :, ky, 0].T
        blob16[0:64, o + 64 : o + 128] = wv[:, :, ky, 0].T
        blob16[64:128, o : o + 64] = wk[:, :, ky, 1].T
        blob16[64:128, o + 64 : o + 128] = wv[:, :, ky, 1].T
        o = C_WKV2 + 128 * ky
        blob16[64:128, o : o + 64] = wk[:, :, ky, 2].T
        blob16[64:128, o + 64 : o + 128] = wv[:, :, ky, 2].T
    blob16[64:128, C_ID : C_ID + 64] = np.eye(64, dtype=f32)

    # MLP weights
    m1w = np.asarray(mlp_w1, f32)
    m1b = np.asarray(mlp_b1, f32)
    m2w = np.asarray(mlp_w2, f32)
    m2b = np.asarray(mlp_b2, f32)
    rel_cell = cell[0] * np.array([HF, WF], f32)
    b1pp = m1b + m1w[:, 64:66] @ rel_cell
    blob16[0:32, C_W1 : C_W1 + 256] = m1w[:, 0:32].T
    blob16[32:64, C_W1 : C_W1 + 256] = m1w[:, 32:64].T
    blob16[64, C_W1 : C_W1 + 256] = b1pp
    blob16[:, C_W2 : C_W2 + 64] = m2w[:, 0:128].T
    blob16[:, C_W2 + 64 : C_W2 + 128] = m2w[:, 128:256].T

    # position-bias table -> multiplicative exp table with window mask
    w1 = np.asarray(cpb_w1, f32)
    b1 = np.asarray(cpb_b1, f32)
    w2c = np.asarray(cpb_w2, f32)
    dy = (np.linspace(-R, R, 2 * R + 1).astype(f32)) * f32(2.0 / HF)
    delta = np.stack(np.meshgrid(dy, dy, indexing="ij"), -1).reshape(-1, 2)
    pb = np.zeros((HEAD, 2, 2, 7, 7), f32)
    for iy in range(2):
        for jx in range(2):
            sy = f32(-1.0) + (2 * iy + 1) / f32(HQ)
            sx = f32(-1.0) + (2 * jx + 1) / f32(WQ)
            base = np.array([f32(-1.0) + 1.0 / f32(HF), f32(-1.0) + 1.0 / f32(WF)], f32)
            ck = base[None, :] + delta
            rel = (np.array([sy, sx], f32)[None, :] - ck) * np.array([HQ, WQ], f32)
            h = np.maximum(rel @ w1.T + b1, 0.0)
            p = h @ w2c.T
            pb[:, iy, jx] = p.T.reshape(HEAD, 7, 7)

    exptab = np.zeros((BKP, 128), f32)
    for kp in range(BKP):
        xk, py = kp // PROWS, kp % PROWS
        for n in range(128):
            h, r = n // 64, n % 64
            qy, qx = r // 4, r % 4
            dyy = py - (qy // 2 + R)
            dxx = xk - (qx // 2 + R)
            if abs(dyy) <= R and abs(dxx) <= R:
                exptab[kp, n] = np.exp(pb[h, qy % 2, qx % 2, dyy + R, dxx + R])
    blob16[0:BKP, C_EXP : C_EXP + 512] = np.tile(exptab, (1, 4))

    blob32 = np.zeros((128, 2), f32)
    blob32[0:64, 0] = np.asarray(conv_q_b, f32) / np.sqrt(f32(DH))
    blob32[0:64, 1] = np.asarray(conv_k_b, f32)
    blob32[64:128, 1] = np.asarray(conv_v_b, f32)

    # padded global maps
    dpad = np.zeros((64, HQ + 2, WQ + 2), f32)
    dpad[:, 1 : 1 + HQ, 1 : 1 + WQ] = depth_T
    fpad = np.zeros((64, HF + 9, HF + 7), f32)
    fpad[:, 4 : 4 + HF, 3 : 3 + WF] = feat_T
    bidx = np.arange(NBLK)[:, None] * 4 + np.arange(6)[None, :]  # (32, 6)

    in_maps = []
    blob16 = blob16.astype(BF)
    for t in range(N_CORES):
        base = dpad[:, 16 * t : 16 * t + 18, :]          # (64, 18, 130)
        d2a = base[:, 0:17, :][:, :, bidx]               # (64, 17, 32, 6)
        d2b = base[:, 1:18, :][:, :, bidx]
        d2blk = np.concatenate([d2a.transpose(0, 2, 1, 3),
                                d2b.transpose(0, 2, 1, 3)], 0)
        f2a = fpad[:, 8 * t : 8 * t + 16, 0:PW].transpose(0, 2, 1)  # (64, 70, 16)
        f2b = fpad[:, 8 * t : 8 * t + 16, 1 : 1 + PW].transpose(0, 2, 1)
        f2d = np.concatenate([f2a, f2b], 0)

        mrow = np.zeros((PROWS,), f32)
        for py in range(PROWS):
            if 0 <= FROWS * t - R + py < HF:
                mrow[py] = 1.0
        b16 = blob16.copy()
        b16[:, C_MSK : C_MSK + 448] = np.tile(mrow, (128, 32)).astype(BF)

        dres_rows = depth[0, QCORE * t : QCORE * (t + 1), :] + m2b[None, :]
        dres3 = dres_rows[_QPERM].T

        m = dict(blob16=np.ascontiguousarray(b16),
                 blob32=blob32,
                 d2blk=np.ascontiguousarray(d2blk.astype(BF)),
                 f2d=np.ascontiguousarray(f2d.astype(BF)),
                 dres3=np.ascontiguousarray(dres3.astype(f32)))
        in_maps.append(m)
    return in_maps


LAST_RESULT = None


def _prep_inputs(inputs):
    keys = dict(inputs)
    keys.pop("shape_y", None)
    keys.pop("shape_x", None)
    return _host_prep(**keys)


def kernel(**inputs):
    global LAST_RESULT
    from concourse.bass_utils import run_bass_kernel_spmd

    if "nc" not in _CACHE:
        _CACHE["nc"] = _build_program()
    nc = _CACHE["nc"]
    in_maps = _prep_inputs(inputs)
    res = run_bass_kernel_spmd(nc, in_maps, core_ids=list(range(N_CORES)))
    LAST_RESULT = res
    parts = []
    for t in range(N_CORES):
        oc = np.ascontiguousarray(np.asarray(res.results[t]["out"]).reshape(64, QCORE).T)
        orow = np.empty_like(oc)
        orow[_QPERM] = oc
        parts.append(orow)
    out = np.concatenate(parts, 0)
    return out[None].astype(np.float32)


def _patch_sim_gelu():
    import math
    import concourse.bass_interp as bi
    import concourse.mybir as mb
    if getattr(bi.InstructionExecutor, "_gelu_patched", False):
        return
    orig = bi.InstructionExecutor.visit_InstActivation
    from concourse.bass_interp import Direction

    erf = np.vectorize(math.erf)

    def patched(self, instruction, *, reg_snapshot=None):
        if getattr(instruction, "func", None) == mb.ActivationFunctionType.Gelu:
            instruction.func = mb.ActivationFunctionType.Identity
            try:
                r = orig(self, instruction, reg_snapshot=reg_snapshot)
            finally:
                instruction.func = mb.ActivationFunctionType.Gelu
            ov = self.view_ap(instruction.outs[0], Direction.WRITE, instruction,
                              reg_snapshot=reg_snapshot)
            x = np.asarray(ov[:], dtype=np.float64)
            ov[:] = (0.5 * x * (1.0 + erf(x / np.sqrt(2.0)))).astype(np.float32)
            return r
        return orig(self, instruction, reg_snapshot=reg_snapshot)

    bi.InstructionExecutor.visit_InstActivation = patched
    bi.InstructionExecutor._gelu_patched = True


def simulate_core(core=0, inputs=None, dbg=False):
    """CoreSim single-core check helper (dev only)."""
    from concourse.bass_interp import CoreSim

    _patch_sim_gelu()

    if "nc" not in _CACHE:
        _CACHE["nc"] = _build_program()
    nc = _CACHE["nc"]
    in_maps = _prep_inputs(inputs)
    sim = CoreSim(nc, trace=False)
    for k, v in in_maps[core].items():
        sim.tensor(k)[:] = v
    sim.simulate(check_with_hw=False)
    return np.ascontiguousarray(np.array(sim.tensor("out")).reshape(64, QCORE).T)
